# revision 1
# baseline (speedup 1.0000x reference)
"""Trainium2 Bass kernel for nn_ConvMatrix2d (CapsNet-style matrix-capsule conv, k=1, s=1).

Computation (per batch b, input-capsule c, spatial position ji = J*14+I):
    out[b, c, o*196 + ji, p*4+r] = sum_q W[c,o,p,q] * x[b,c,I,J,q*4+r]
    out[b, c, o*196 + ji, 16]    = x[b,c,I,J,16]
Output: (32, 32, 6272, 17) fp32 = 437 MB  -> heavily output-DMA bound.

Strategy (8 cores, data parallel over batch: 4 batches/core):
  - Host packs x into per-(b,c) moving operands x2[5, 784] (rows q of pose in
    (ji, r) order + act row replicated x4) and weights into stationary
    operands w2[c][5, 160] (4 p-blocks of 32 cols + 32 act columns).
  - Device, per (b, c_hi) (c = c_hi*4 + c_lo): 4-way col/row-tiled matmuls
    (K=4/5, M=32 at array position c_lo*32) emit V_p[o, (ji,r)] into PSUM;
    DVE/ACT interleave-copy into a staging tile [128 = (c_lo, o), 3332 =
    (ji, t)] which is exactly HBM layout; one 1.7MB out-DMA per (b, c_hi)
    with 13.3KB-contiguous descriptors across all 128 partitions.
"""

import numpy as np

import concourse.bass as bass
import concourse.bacc as bacc
import concourse.mybir as mybir
from concourse.tile import TileContext
from concourse.bass_utils import run_bass_kernel_spmd

# Problem constants (hardcoded per contract)
B, C, WSP, HH = 32, 32, 14, 17
O, H = 32, 4
JI = WSP * WSP          # 196
NB = 4                  # batches per core
NCORES = 8
CHI, CLO = 8, 4         # c = c_hi*4 + c_lo
NQ = 4                  # ji quarters of 49
QJ = 49                 # ji per quarter
FQ = QJ * 4             # 196 moving-free elems per quarter (ji x r)
ROW = HH                # 17 floats per output row
SLAB = JI * HH          # 3332 floats per (b,c,o)

F32 = mybir.dt.float32


def _build_nc():
    nc = bacc.Bacc()
    x_d = nc.dram_tensor("x2", [NB, CHI, CLO, 5, 784], F32, kind="ExternalInput")
    w_d = nc.dram_tensor("w2", [CLO, 5, CHI, 160], F32, kind="ExternalInput")
    out_d = nc.dram_tensor("out", [NB, C, O * JI, HH], F32, kind="ExternalOutput")

    with TileContext(nc) as tc:
        with (
            tc.tile_pool(name="wpool", bufs=1) as wpool,
            tc.tile_pool(name="xpool", bufs=3) as xpool,
            tc.tile_pool(name="stage", bufs=3) as spool,
            tc.tile_pool(name="psv", bufs=3, space="PSUM") as pv_pool,
            tc.tile_pool(name="psa", bufs=2, space="PSUM") as pa_pool,
        ):
            # Resident weights: partitions {c_lo*32 + k : k<5}, free = c_hi*160 + col
            # NB: one dma_start per 32-partition block — SBUF-side DMA APs only
            # support partition crossing via dim 0 (inner partition-step dims
            # get flat-merged and corrupt memory on HW).
            w_sb = wpool.tile([128, CHI * 160], F32)
            for c_lo in range(CLO):
                nc.sync.dma_start(
                    out=w_sb[c_lo * 32: c_lo * 32 + 5, :],
                    in_=w_d[c_lo],
                )

            for b in range(NB):
                for c_hi in range(CHI):
                    x_sb = xpool.tile([128, 784], F32, tag="x")
                    for c_lo in range(CLO):
                        nc.sync.dma_start(
                            out=x_sb[c_lo * 32: c_lo * 32 + 5, :],
                            in_=x_d[b, c_hi, c_lo],
                        )
                    stage = spool.tile([128, SLAB], F32, tag="stage")

                    for quarter in range(NQ):
                        # PSUM tiles shared by the 4 c_lo col-groups
                        vt = pv_pool.tile([128, 1024], F32, tag="v")
                        at = pa_pool.tile([128, FQ], F32, tag="a")
                        for p in range(4):
                            for c_lo in range(CLO):
                                pbase = c_lo * 32
                                lhsT = w_sb[pbase:pbase + 4,
                                            c_hi * 160 + p * 32: c_hi * 160 + (p + 1) * 32]
                                rhs = x_sb[pbase:pbase + 4,
                                           quarter * FQ:(quarter + 1) * FQ]
                                nc.tensor.matmul(
                                    vt[pbase:pbase + 32, p * 256: p * 256 + FQ],
                                    lhsT, rhs,
                                    tile_position=(pbase, pbase),
                                )
                        for c_lo in range(CLO):
                            pbase = c_lo * 32
                            lhsT = w_sb[pbase:pbase + 5,
                                        c_hi * 160 + 128: c_hi * 160 + 160]
                            rhs = x_sb[pbase:pbase + 5,
                                       quarter * FQ:(quarter + 1) * FQ]
                            nc.tensor.matmul(at[pbase:pbase + 32, :], lhsT, rhs,
                                             tile_position=(pbase, pbase))

                        # Interleave-copy PSUM -> staging rows (ji*17 + t)
                        # votes: src [128][p 4 step 256][196 contig]
                        #        dst [128][p 4 step 4][jj 49 step 17][r 4 step 1]
                        qbase = quarter * QJ * ROW
                        for p in range(4):
                            src = vt.rearrange("z (jj r) -> z jj r", jj=4 * 64)[
                                :, p * 64: p * 64 + QJ, :]
                            dst = stage.rearrange("z (ji t) -> z ji t", t=ROW)[
                                :, quarter * QJ:(quarter + 1) * QJ, p * 4: p * 4 + 4]
                            if p != 3:
                                nc.vector.tensor_copy(dst, src)
                            else:
                                nc.scalar.copy(dst, src)
                        # act: src r=0 slice [128][jj 49 step 4]; dst [128][jj step 17] at t=16
                        asrc = at.rearrange("z (jj r) -> z jj r", r=4)[:, :, 0]
                        adst = stage.rearrange("z (ji t) -> z ji t", t=ROW)[
                            :, quarter * QJ:(quarter + 1) * QJ, 16]
                        nc.vector.tensor_copy(adst, asrc)

                    # One 1.7MB out-DMA: dst [c_lo 4][o 32][3332 contig]
                    dst = out_d.rearrange(
                        "b (ch cl) (o j) t -> b ch cl o (j t)", cl=CLO, o=O
                    )[b, c_hi]
                    nc.sync.dma_start(out=dst, in_=stage[:])
    if not nc.is_finalized():
        nc.finalize()
    return nc


_CACHE = {}


def _get_nc():
    if "nc" not in _CACHE:
        _CACHE["nc"] = _build_nc()
    return _CACHE["nc"]


def _preprocess(x, weight):
    """Build per-core input maps from full inputs."""
    x = np.ascontiguousarray(x, dtype=np.float32)
    weight = np.ascontiguousarray(weight, dtype=np.float32)
    xp = x.transpose(0, 1, 3, 2, 4).reshape(B, C, JI, HH)  # ji = J*14+I
    x2 = np.empty((B, C, 5, 784), dtype=np.float32)
    pose = xp[..., :16].reshape(B, C, JI, 4, 4)
    x2[:, :, :4, :] = pose.transpose(0, 1, 3, 2, 4).reshape(B, C, 4, 784)
    x2[:, :, 4, :] = np.repeat(xp[..., 16], 4, axis=-1).reshape(B, C, 784)
    # device layout: (b, c_hi, c_lo, 5, 784)
    x2 = x2.reshape(B, CHI, CLO, 5, 784)

    Wm = weight[:, 0, 0]  # (C, O, 4, 4): W[c,o,p,q]
    w2 = np.zeros((C, 5, 160), dtype=np.float32)
    for p in range(4):
        w2[:, :4, p * 32:(p + 1) * 32] = Wm[:, :, p, :].transpose(0, 2, 1)
    w2[:, 4, 128:160] = 1.0
    # device layout: (c_lo, 5, c_hi, 160)
    w2 = np.ascontiguousarray(
        w2.reshape(CHI, CLO, 5, 160).transpose(1, 2, 0, 3))

    in_maps = []
    for k in range(NCORES):
        in_maps.append({
            "x2": np.ascontiguousarray(x2[k * NB:(k + 1) * NB]),
            "w2": w2,
        })
    return in_maps


def _run(x, weight, trace=False, trace_kwargs=None):
    nc = _get_nc()
    in_maps = _preprocess(x, weight)
    res = run_bass_kernel_spmd(
        nc, in_maps, list(range(NCORES)), trace=trace,
        trace_kwargs=trace_kwargs or {},
    )
    out = np.concatenate([r["out"] for r in res.results], axis=0)
    return out.astype(np.float32, copy=False), res


def kernel(x, weight):
    out, _ = _run(x, weight)
    return out



# revision 4
# speedup vs baseline: 2.3304x; 2.3304x over previous
"""Trainium2 Bass kernel for nn_ConvMatrix2d (CapsNet-style matrix-capsule conv, k=1, s=1).

Computation (per batch b, input-capsule c, spatial position ji = J*14+I):
    out[b, c, o*196 + ji, p*4+r] = sum_q W[c,o,p,q] * x[b,c,I,J,q*4+r]
    out[b, c, o*196 + ji, 16]    = x[b,c,I,J,16]
Output: (32, 32, 6272, 17) fp32 = 437 MB  -> heavily output-DMA bound.

Strategy (8 cores, data parallel over batch: 4 batches/core):
  - Votes for 4 channels (c_lo) at once via ONE block-diagonal matmul:
    lhsT[(c_lo,q), (c_lo',o)] = W[c,o,p,q] iff c_lo==c_lo' (16x128, zeros
    elsewhere), rhs[(c_lo,q), (ji,r)] = pose rows of the 4 channels. That
    fills all 128 output partitions (c_lo,o) from a single rhs stream, so
    the PE streams the minimum number of moving columns.
  - float32r matmuls (1 cycle/col at N>=392 vs 4 for fp32) -> tensor
    engine drops from ~416us to <100us per core; output DMA (54.7 MB @
    ~358 GB/s) becomes the bottleneck.
  - Acts broadcast over o via a tiny K=4 delta-matmul from partition
    strip 1 (rows 32..36).
  - DVE/ACT interleave-copy PSUM -> staging tile [128=(c_lo,o), 3332=
    (ji,t)] which is exactly HBM layout; one 1.7MB out-DMA per (b, c_hi)
    with 13.3KB-contiguous descriptors across all 128 partitions.
"""

import numpy as np

import concourse.bass as bass
import concourse.bacc as bacc
import concourse.mybir as mybir
from concourse.tile import TileContext
from concourse.bass_utils import run_bass_kernel_spmd

# Problem constants (hardcoded per contract)
B, C, WSP, HH = 32, 32, 14, 17
O, H = 32, 4
JI = WSP * WSP          # 196
NB = 4                  # batches per core
NCORES = 8
CHI, CLO = 8, 4         # c = c_hi*4 + c_lo
HJ = 98                 # ji per half
FH = HJ * 4             # 392 moving cols per half (ji x r)
ROW = HH                # 17 floats per output row
SLAB = JI * HH          # 3332 floats per (b,c,o)

F32 = mybir.dt.float32
F32R = mybir.dt.float32r


def _build_nc():
    nc = bacc.Bacc()
    x_d = nc.dram_tensor("x3", [NB, CHI, 20, 784], F32R, kind="ExternalInput")
    w_d = nc.dram_tensor("w3", [16, CHI * 4 * 128], F32R, kind="ExternalInput")
    wa_d = nc.dram_tensor("wact", [4, 128], F32R, kind="ExternalInput")
    out_d = nc.dram_tensor("out", [NB, C, O * JI, HH], F32, kind="ExternalOutput")

    with TileContext(nc) as tc:
        with (
            tc.tile_pool(name="wpool", bufs=1) as wpool,
            tc.tile_pool(name="xpool", bufs=3) as xpool,
            tc.tile_pool(name="stage", bufs=3) as spool,
            tc.tile_pool(name="psv", bufs=6, space="PSUM") as pv_pool,
            tc.tile_pool(name="psa", bufs=2, space="PSUM") as pa_pool,
        ):
            # Resident weights.
            # w_sb rows 0..16 = (c_lo, q); free = (c_hi, p, 128 block-diag cols)
            w_sb = wpool.tile([16, CHI * 4 * 128], F32R)
            nc.sync.dma_start(out=w_sb[:, :], in_=w_d[:, :])
            # delta weights for act broadcast, on partition strip 1
            wact_sb = wpool.tile([128, 128], F32R)
            nc.sync.dma_start(out=wact_sb[32:36, :], in_=wa_d[:, :])

            for b in range(NB):
                for c_hi in range(CHI):
                    x_sb = xpool.tile([128, 784], F32R, tag="x")
                    # pose rows (c_lo,q) at partitions 0..16
                    nc.sync.dma_start(out=x_sb[0:16, :], in_=x_d[b, c_hi, 0:16])
                    # act rows (c_lo) at partitions 32..36 (strip 1)
                    nc.sync.dma_start(out=x_sb[32:36, :], in_=x_d[b, c_hi, 16:20])
                    stage = spool.tile([128, SLAB], F32, tag="stage")

                    for half in range(2):
                        cl, ch = half * FH, (half + 1) * FH
                        for p in range(4):
                            vt = pv_pool.tile([128, FH], F32, tag="v")
                            lhsT = w_sb[0:16,
                                        (c_hi * 4 + p) * 128:(c_hi * 4 + p + 1) * 128]
                            nc.tensor.matmul(
                                vt[:, :],
                                lhsT,
                                x_sb[0:16, cl:ch],
                            )
                            # Interleave-copy PSUM -> staging rows (ji*17 + p*4 + r)
                            src = vt.rearrange("z (jj r) -> z jj r", r=4)
                            dst = stage.rearrange("z (ji t) -> z ji t", t=ROW)[
                                :, half * HJ:(half + 1) * HJ, p * 4: p * 4 + 4]
                            if p < 2:
                                nc.vector.tensor_copy(dst, src)
                            else:
                                nc.scalar.copy(dst, src)

                        at = pa_pool.tile([128, FH], F32, tag="a")
                        nc.tensor.matmul(
                            at[:, :],
                            wact_sb[32:36, :],
                            x_sb[32:36, cl:ch],
                        )
                        asrc = at.rearrange("z (jj r) -> z jj r", r=4)[:, :, 0]
                        adst = stage.rearrange("z (ji t) -> z ji t", t=ROW)[
                            :, half * HJ:(half + 1) * HJ, 16]
                        nc.vector.tensor_copy(adst, asrc)

                    # One 1.7MB out-DMA: dst [c_lo 4][o 32][3332 contig]
                    dst = out_d.rearrange(
                        "b (ch cl) (o j) t -> b ch cl o (j t)", cl=CLO, o=O
                    )[b, c_hi]
                    nc.sync.dma_start(out=dst, in_=stage[:])
    if not nc.is_finalized():
        nc.finalize()
    return nc


_CACHE = {}


def _get_nc():
    if "nc" not in _CACHE:
        _CACHE["nc"] = _build_nc()
    return _CACHE["nc"]


def _preprocess(x, weight):
    """Build per-core input maps from full inputs."""
    x = np.ascontiguousarray(x, dtype=np.float32)
    weight = np.ascontiguousarray(weight, dtype=np.float32)
    xp = x.transpose(0, 1, 3, 2, 4).reshape(B, C, JI, HH)  # ji = J*14+I
    # x3[b, c_hi, row, (ji,r)]: rows 0..16 = (c_lo, q) pose, 16..20 = act x4
    x3 = np.empty((B, C, 5, 784), dtype=np.float32)
    pose = xp[..., :16].reshape(B, C, JI, 4, 4)            # [b,c,ji,q,r]
    x3[:, :, :4, :] = pose.transpose(0, 1, 3, 2, 4).reshape(B, C, 4, 784)
    x3[:, :, 4, :] = np.repeat(xp[..., 16], 4, axis=-1).reshape(B, C, 784)
    # rows: (c_lo, q) pose block then (c_lo) act block
    x3 = x3.reshape(B, CHI, CLO, 5, 784)
    x3p = np.empty((B, CHI, 20, 784), dtype=np.float32)
    x3p[:, :, :16] = x3[:, :, :, :4].reshape(B, CHI, 16, 784)
    x3p[:, :, 16:] = x3[:, :, :, 4]

    Wm = weight[:, 0, 0]                                   # (C, O, 4, 4): W[c,o,p,q]
    w3 = np.zeros((16, CHI, 4, 128), dtype=np.float32)
    WmB = Wm.reshape(CHI, CLO, O, 4, 4)                    # [c_hi, c_lo, o, p, q]
    for c_lo in range(CLO):
        # dst w3[c_lo*4+q, c_hi, p, c_lo*32+o]
        w3[c_lo * 4:c_lo * 4 + 4, :, :, c_lo * 32:c_lo * 32 + 32] = (
            WmB[:, c_lo].transpose(3, 0, 2, 1))            # (q, c_hi, p, o)
    w3 = np.ascontiguousarray(w3.reshape(16, CHI * 4 * 128))

    wact = np.zeros((4, 128), dtype=np.float32)
    for c_lo in range(CLO):
        wact[c_lo, c_lo * 32:(c_lo + 1) * 32] = 1.0

    in_maps = []
    for k in range(NCORES):
        in_maps.append({
            "x3": np.ascontiguousarray(x3p[k * NB:(k + 1) * NB]),
            "w3": w3,
            "wact": wact,
        })
    return in_maps


def _run(x, weight, trace=False, trace_kwargs=None):
    nc = _get_nc()
    in_maps = _preprocess(x, weight)
    res = run_bass_kernel_spmd(
        nc, in_maps, list(range(NCORES)), trace=trace,
        trace_kwargs=trace_kwargs or {},
    )
    out = np.concatenate([r["out"] for r in res.results], axis=0)
    return out.astype(np.float32, copy=False), res


def kernel(x, weight):
    out, _ = _run(x, weight)
    return out


# revision 5
# speedup vs baseline: 2.8560x; 1.2255x over previous
"""Trainium2 Bass kernel for nn_ConvMatrix2d (CapsNet-style matrix-capsule conv, k=1, s=1).

Computation (per batch b, input-capsule c, spatial position ji = J*14+I):
    out[b, c, o*196 + ji, p*4+r] = sum_q W[c,o,p,q] * x[b,c,I,J,q*4+r]
    out[b, c, o*196 + ji, 16]    = x[b,c,I,J,16]
Output: (32, 32, 6272, 17) fp32 = 437 MB  -> heavily output-DMA bound.

Strategy (8 cores, data parallel over batch: 4 batches/core):
  - Votes for 4 channels (c_lo) at once via ONE block-diagonal matmul:
    lhsT[(c_lo,q), (c_lo',o)] = W[c,o,p,q] iff c_lo==c_lo' (16x128, zeros
    elsewhere), rhs[(c_lo,q), (ji,r)] = pose rows of the 4 channels. That
    fills all 128 output partitions (c_lo,o) from a single rhs stream, so
    the PE streams the minimum number of moving columns.
  - float32r matmuls (1 cycle/col at N>=392 vs 4 for fp32) -> tensor
    engine drops from ~416us to <100us per core; output DMA (54.7 MB @
    ~358 GB/s) becomes the bottleneck.
  - Acts broadcast over o via a tiny K=4 delta-matmul from partition
    strip 1 (rows 32..36).
  - DVE/ACT interleave-copy PSUM -> staging tile [128=(c_lo,o), 3332=
    (ji,t)] which is exactly HBM layout; one 1.7MB out-DMA per (b, c_hi)
    with 13.3KB-contiguous descriptors across all 128 partitions.
"""

import numpy as np

import concourse.bass as bass
import concourse.bacc as bacc
import concourse.mybir as mybir
from concourse.tile import TileContext
from concourse.bass_utils import run_bass_kernel_spmd

# Problem constants (hardcoded per contract)
B, C, WSP, HH = 32, 32, 14, 17
O, H = 32, 4
JI = WSP * WSP          # 196
NB = 4                  # batches per core
NCORES = 8
CHI, CLO = 8, 4         # c = c_hi*4 + c_lo
HJ = 98                 # ji per half
FH = HJ * 4             # 392 moving cols per half (ji x r)
ROW = HH                # 17 floats per output row
SLAB = JI * HH          # 3332 floats per (b,c,o)

F32 = mybir.dt.float32
F32R = mybir.dt.float32r


def _build_nc():
    nc = bacc.Bacc()
    x_d = nc.dram_tensor("x3", [NB, CHI, 20, 784], F32R, kind="ExternalInput")
    w_d = nc.dram_tensor("w3", [16, CHI * 4 * 128], F32R, kind="ExternalInput")
    wa_d = nc.dram_tensor("wact", [4, 128], F32R, kind="ExternalInput")
    out_d = nc.dram_tensor("out", [NB, C, O * JI, HH], F32, kind="ExternalOutput")

    with TileContext(nc) as tc:
        with (
            tc.tile_pool(name="wpool", bufs=1) as wpool,
            tc.tile_pool(name="xpool", bufs=4) as xpool,
            tc.tile_pool(name="stage", bufs=4) as spool,
            tc.tile_pool(name="psv", bufs=6, space="PSUM") as pv_pool,
            tc.tile_pool(name="psa", bufs=2, space="PSUM") as pa_pool,
        ):
            # Resident weights.
            # w_sb rows 0..16 = (c_lo, q); free = (c_hi, p, 128 block-diag cols)
            w_sb = wpool.tile([16, CHI * 4 * 128], F32R)
            nc.sync.dma_start(out=w_sb[:, :], in_=w_d[:, :])
            # delta weights for act broadcast, on partition strip 1
            wact_sb = wpool.tile([128, 128], F32R)
            nc.sync.dma_start(out=wact_sb[32:36, :], in_=wa_d[:, :])

            for b in range(NB):
                for c_hi in range(CHI):
                    x_sb = xpool.tile([128, 784], F32R, tag="x")
                    # pose rows (c_lo,q) at partitions 0..16
                    nc.gpsimd.dma_start(out=x_sb[0:16, :], in_=x_d[b, c_hi, 0:16])
                    # act rows (c_lo) at partitions 32..36 (strip 1)
                    nc.gpsimd.dma_start(out=x_sb[32:36, :], in_=x_d[b, c_hi, 16:20])
                    stage = spool.tile([128, SLAB], F32, tag="stage")

                    for half in range(2):
                        cl, ch = half * FH, (half + 1) * FH
                        for p in range(4):
                            vt = pv_pool.tile([128, FH], F32, tag="v")
                            lhsT = w_sb[0:16,
                                        (c_hi * 4 + p) * 128:(c_hi * 4 + p + 1) * 128]
                            nc.tensor.matmul(
                                vt[:, :],
                                lhsT,
                                x_sb[0:16, cl:ch],
                            )
                            # Interleave-copy PSUM -> staging rows (ji*17 + p*4 + r)
                            src = vt.rearrange("z (jj r) -> z jj r", r=4)
                            dst = stage.rearrange("z (ji t) -> z ji t", t=ROW)[
                                :, half * HJ:(half + 1) * HJ, p * 4: p * 4 + 4]
                            if p < 2:
                                nc.vector.tensor_copy(dst, src)
                            else:
                                nc.scalar.copy(dst, src)

                        at = pa_pool.tile([128, FH], F32, tag="a")
                        nc.tensor.matmul(
                            at[:, :],
                            wact_sb[32:36, :],
                            x_sb[32:36, cl:ch],
                        )
                        asrc = at.rearrange("z (jj r) -> z jj r", r=4)[:, :, 0]
                        adst = stage.rearrange("z (ji t) -> z ji t", t=ROW)[
                            :, half * HJ:(half + 1) * HJ, 16]
                        nc.vector.tensor_copy(adst, asrc)

                    # One 1.7MB out-DMA: dst [c_lo 4][o 32][3332 contig]
                    dst = out_d.rearrange(
                        "b (ch cl) (o j) t -> b ch cl o (j t)", cl=CLO, o=O
                    )[b, c_hi]
                    nc.sync.dma_start(out=dst, in_=stage[:])
    if not nc.is_finalized():
        nc.finalize()
    return nc


_CACHE = {}


def _get_nc():
    if "nc" not in _CACHE:
        _CACHE["nc"] = _build_nc()
    return _CACHE["nc"]


def _preprocess(x, weight):
    """Build per-core input maps from full inputs."""
    x = np.ascontiguousarray(x, dtype=np.float32)
    weight = np.ascontiguousarray(weight, dtype=np.float32)
    xp = x.transpose(0, 1, 3, 2, 4).reshape(B, C, JI, HH)  # ji = J*14+I
    # x3[b, c_hi, row, (ji,r)]: rows 0..16 = (c_lo, q) pose, 16..20 = act x4
    x3 = np.empty((B, C, 5, 784), dtype=np.float32)
    pose = xp[..., :16].reshape(B, C, JI, 4, 4)            # [b,c,ji,q,r]
    x3[:, :, :4, :] = pose.transpose(0, 1, 3, 2, 4).reshape(B, C, 4, 784)
    x3[:, :, 4, :] = np.repeat(xp[..., 16], 4, axis=-1).reshape(B, C, 784)
    # rows: (c_lo, q) pose block then (c_lo) act block
    x3 = x3.reshape(B, CHI, CLO, 5, 784)
    x3p = np.empty((B, CHI, 20, 784), dtype=np.float32)
    x3p[:, :, :16] = x3[:, :, :, :4].reshape(B, CHI, 16, 784)
    x3p[:, :, 16:] = x3[:, :, :, 4]

    Wm = weight[:, 0, 0]                                   # (C, O, 4, 4): W[c,o,p,q]
    w3 = np.zeros((16, CHI, 4, 128), dtype=np.float32)
    WmB = Wm.reshape(CHI, CLO, O, 4, 4)                    # [c_hi, c_lo, o, p, q]
    for c_lo in range(CLO):
        # dst w3[c_lo*4+q, c_hi, p, c_lo*32+o]
        w3[c_lo * 4:c_lo * 4 + 4, :, :, c_lo * 32:c_lo * 32 + 32] = (
            WmB[:, c_lo].transpose(3, 0, 2, 1))            # (q, c_hi, p, o)
    w3 = np.ascontiguousarray(w3.reshape(16, CHI * 4 * 128))

    wact = np.zeros((4, 128), dtype=np.float32)
    for c_lo in range(CLO):
        wact[c_lo, c_lo * 32:(c_lo + 1) * 32] = 1.0

    in_maps = []
    for k in range(NCORES):
        in_maps.append({
            "x3": np.ascontiguousarray(x3p[k * NB:(k + 1) * NB]),
            "w3": w3,
            "wact": wact,
        })
    return in_maps


def _run(x, weight, trace=False, trace_kwargs=None):
    nc = _get_nc()
    in_maps = _preprocess(x, weight)
    res = run_bass_kernel_spmd(
        nc, in_maps, list(range(NCORES)), trace=trace,
        trace_kwargs=trace_kwargs or {},
    )
    out = np.concatenate([r["out"] for r in res.results], axis=0)
    return out.astype(np.float32, copy=False), res


def kernel(x, weight):
    out, _ = _run(x, weight)
    return out


# revision 7
# speedup vs baseline: 2.8674x; 1.0040x over previous
"""Trainium2 Bass kernel for nn_ConvMatrix2d (CapsNet-style matrix-capsule conv, k=1, s=1).

Computation (per batch b, input-capsule c, spatial position ji = J*14+I):
    out[b, c, o*196 + ji, p*4+r] = sum_q W[c,o,p,q] * x[b,c,I,J,q*4+r]
    out[b, c, o*196 + ji, 16]    = x[b,c,I,J,16]
Output: (32, 32, 6272, 17) fp32 = 437 MB  -> heavily output-DMA bound.

Strategy (8 cores, data parallel over batch: 4 batches/core):
  - Votes for 4 channels (c_lo) at once via ONE block-diagonal matmul:
    lhsT[(c_lo,q), (c_lo',o)] = W[c,o,p,q] iff c_lo==c_lo' (16x128, zeros
    elsewhere), rhs[(c_lo,q), (ji,r)] = pose rows of the 4 channels. That
    fills all 128 output partitions (c_lo,o) from a single rhs stream, so
    the PE streams the minimum number of moving columns.
  - float32r matmuls (1 cycle/col at N>=392 vs 4 for fp32) -> tensor
    engine drops from ~416us to <100us per core; output DMA (54.7 MB @
    ~358 GB/s) becomes the bottleneck.
  - Acts broadcast over o via a tiny K=4 delta-matmul from partition
    strip 1 (rows 32..36).
  - DVE/ACT interleave-copy PSUM -> staging tile [128=(c_lo,o), 3332=
    (ji,t)] which is exactly HBM layout; one 1.7MB out-DMA per (b, c_hi)
    with 13.3KB-contiguous descriptors across all 128 partitions.
"""

import numpy as np

import concourse.bass as bass
import concourse.bacc as bacc
import concourse.mybir as mybir
from concourse.tile import TileContext
from concourse.bass_utils import run_bass_kernel_spmd

# Problem constants (hardcoded per contract)
B, C, WSP, HH = 32, 32, 14, 17
O, H = 32, 4
JI = WSP * WSP          # 196
NB = 4                  # batches per core
NCORES = 8
CHI, CLO = 8, 4         # c = c_hi*4 + c_lo
HJ = 98                 # ji per half
FH = HJ * 4             # 392 moving cols per half (ji x r)
ROW = HH                # 17 floats per output row
SLAB = JI * HH          # 3332 floats per (b,c,o)

F32 = mybir.dt.float32
F32R = mybir.dt.float32r


def _build_nc():
    nc = bacc.Bacc()
    x_d = nc.dram_tensor("x3", [NB, CHI, 20, 784], F32R, kind="ExternalInput")
    w_d = nc.dram_tensor("w3", [16, CHI * 4 * 128], F32R, kind="ExternalInput")
    wa_d = nc.dram_tensor("wact", [4, 128], F32R, kind="ExternalInput")
    out_d = nc.dram_tensor("out", [NB, C, O * JI, HH], F32, kind="ExternalOutput")

    with TileContext(nc) as tc:
        with (
            tc.tile_pool(name="wpool", bufs=1) as wpool,
            tc.tile_pool(name="xpool", bufs=4) as xpool,
            tc.tile_pool(name="stage", bufs=6) as spool,
            tc.tile_pool(name="psv", bufs=6, space="PSUM") as pv_pool,
            tc.tile_pool(name="psa", bufs=2, space="PSUM") as pa_pool,
        ):
            # Resident weights.
            # w_sb rows 0..16 = (c_lo, q); free = (c_hi, p, 128 block-diag cols)
            w_sb = wpool.tile([16, CHI * 4 * 128], F32R)
            nc.sync.dma_start(out=w_sb[:, :], in_=w_d[:, :])
            # delta weights for act broadcast, on partition strip 1
            wact_sb = wpool.tile([128, 128], F32R)
            nc.sync.dma_start(out=wact_sb[32:36, :], in_=wa_d[:, :])

            HSLAB = HJ * ROW    # 1666 floats per half-slab row
            for b in range(NB):
                for c_hi in range(CHI):
                    x_sb = xpool.tile([128, 784], F32R, tag="x")
                    # pose rows (c_lo,q) at partitions 0..16  (HWDGE/ACT ring)
                    nc.scalar.dma_start(out=x_sb[0:16, :], in_=x_d[b, c_hi, 0:16])
                    # act rows (c_lo) at partitions 32..36 (strip 1)
                    nc.scalar.dma_start(out=x_sb[32:36, :], in_=x_d[b, c_hi, 16:20])

                    for half in range(2):
                        cl, ch = half * FH, (half + 1) * FH
                        stage = spool.tile([128, HSLAB], F32, tag="stage")
                        sview = stage.rearrange("z (ji t) -> z ji t", t=ROW)

                        at = pa_pool.tile([128, FH], F32, tag="a")
                        nc.tensor.matmul(
                            at[:, :],
                            wact_sb[32:36, :],
                            x_sb[32:36, cl:ch],
                        )
                        asrc = at.rearrange("z (jj r) -> z jj r", r=4)[:, :, 0]
                        nc.vector.tensor_copy(sview[:, :, 16], asrc)

                        for p in range(4):
                            vt = pv_pool.tile([128, FH], F32, tag="v")
                            lhsT = w_sb[0:16,
                                        (c_hi * 4 + p) * 128:(c_hi * 4 + p + 1) * 128]
                            nc.tensor.matmul(
                                vt[:, :],
                                lhsT,
                                x_sb[0:16, cl:ch],
                            )
                            # Interleave-copy PSUM -> staging rows (ji*17 + p*4 + r)
                            src = vt.rearrange("z (jj r) -> z jj r", r=4)
                            dst = sview[:, :, p * 4: p * 4 + 4]
                            if p < 2:
                                nc.vector.tensor_copy(dst, src)
                            else:
                                nc.scalar.copy(dst, src)

                        # 0.85MB half out-DMA: dst [c_lo 4][o 32][1666 contig]
                        dst = out_d.rearrange(
                            "b (ch cl) (o j) t -> b ch cl o (j t)", cl=CLO, o=O
                        )[b, c_hi, :, :, half * HSLAB:(half + 1) * HSLAB]
                        nc.sync.dma_start(out=dst, in_=stage[:])
    if not nc.is_finalized():
        nc.finalize()
    return nc


_CACHE = {}


def _get_nc():
    if "nc" not in _CACHE:
        _CACHE["nc"] = _build_nc()
    return _CACHE["nc"]


def _preprocess(x, weight):
    """Build per-core input maps from full inputs."""
    x = np.ascontiguousarray(x, dtype=np.float32)
    weight = np.ascontiguousarray(weight, dtype=np.float32)
    xp = x.transpose(0, 1, 3, 2, 4).reshape(B, C, JI, HH)  # ji = J*14+I
    # x3[b, c_hi, row, (ji,r)]: rows 0..16 = (c_lo, q) pose, 16..20 = act x4
    x3 = np.empty((B, C, 5, 784), dtype=np.float32)
    pose = xp[..., :16].reshape(B, C, JI, 4, 4)            # [b,c,ji,q,r]
    x3[:, :, :4, :] = pose.transpose(0, 1, 3, 2, 4).reshape(B, C, 4, 784)
    x3[:, :, 4, :] = np.repeat(xp[..., 16], 4, axis=-1).reshape(B, C, 784)
    # rows: (c_lo, q) pose block then (c_lo) act block
    x3 = x3.reshape(B, CHI, CLO, 5, 784)
    x3p = np.empty((B, CHI, 20, 784), dtype=np.float32)
    x3p[:, :, :16] = x3[:, :, :, :4].reshape(B, CHI, 16, 784)
    x3p[:, :, 16:] = x3[:, :, :, 4]

    Wm = weight[:, 0, 0]                                   # (C, O, 4, 4): W[c,o,p,q]
    w3 = np.zeros((16, CHI, 4, 128), dtype=np.float32)
    WmB = Wm.reshape(CHI, CLO, O, 4, 4)                    # [c_hi, c_lo, o, p, q]
    for c_lo in range(CLO):
        # dst w3[c_lo*4+q, c_hi, p, c_lo*32+o]
        w3[c_lo * 4:c_lo * 4 + 4, :, :, c_lo * 32:c_lo * 32 + 32] = (
            WmB[:, c_lo].transpose(3, 0, 2, 1))            # (q, c_hi, p, o)
    w3 = np.ascontiguousarray(w3.reshape(16, CHI * 4 * 128))

    wact = np.zeros((4, 128), dtype=np.float32)
    for c_lo in range(CLO):
        wact[c_lo, c_lo * 32:(c_lo + 1) * 32] = 1.0

    in_maps = []
    for k in range(NCORES):
        in_maps.append({
            "x3": np.ascontiguousarray(x3p[k * NB:(k + 1) * NB]),
            "w3": w3,
            "wact": wact,
        })
    return in_maps


def _run(x, weight, trace=False, trace_kwargs=None):
    nc = _get_nc()
    in_maps = _preprocess(x, weight)
    res = run_bass_kernel_spmd(
        nc, in_maps, list(range(NCORES)), trace=trace,
        trace_kwargs=trace_kwargs or {},
    )
    out = np.concatenate([r["out"] for r in res.results], axis=0)
    return out.astype(np.float32, copy=False), res


def kernel(x, weight):
    out, _ = _run(x, weight)
    return out


# revision 8
# speedup vs baseline: 2.9536x; 1.0300x over previous
"""Trainium2 Bass kernel for nn_ConvMatrix2d (CapsNet-style matrix-capsule conv, k=1, s=1).

Computation (per batch b, input-capsule c, spatial position ji = J*14+I):
    out[b, c, o*196 + ji, p*4+r] = sum_q W[c,o,p,q] * x[b,c,I,J,q*4+r]
    out[b, c, o*196 + ji, 16]    = x[b,c,I,J,16]
Output: (32, 32, 6272, 17) fp32 = 437 MB  -> heavily output-DMA bound.

Strategy (8 cores, data parallel over batch: 4 batches/core):
  - Votes for 4 channels (c_lo) at once via ONE block-diagonal matmul:
    lhsT[(c_lo,q), (c_lo',o)] = W[c,o,p,q] iff c_lo==c_lo' (16x128, zeros
    elsewhere), rhs[(c_lo,q), (ji,r)] = pose rows of the 4 channels. That
    fills all 128 output partitions (c_lo,o) from a single rhs stream, so
    the PE streams the minimum number of moving columns.
  - float32r matmuls (1 cycle/col at N>=392 vs 4 for fp32) -> tensor
    engine drops from ~416us to <100us per core; output DMA (54.7 MB @
    ~358 GB/s) becomes the bottleneck.
  - Acts broadcast over o via a tiny K=4 delta-matmul from partition
    strip 1 (rows 32..36).
  - DVE/ACT interleave-copy PSUM -> staging tile [128=(c_lo,o), 3332=
    (ji,t)] which is exactly HBM layout; one 1.7MB out-DMA per (b, c_hi)
    with 13.3KB-contiguous descriptors across all 128 partitions.
"""

import numpy as np

import concourse.bass as bass
import concourse.bacc as bacc
import concourse.mybir as mybir
from concourse.tile import TileContext
from concourse.bass_utils import run_bass_kernel_spmd

# Problem constants (hardcoded per contract)
B, C, WSP, HH = 32, 32, 14, 17
O, H = 32, 4
JI = WSP * WSP          # 196
NB = 4                  # batches per core
NCORES = 8
CHI, CLO = 8, 4         # c = c_hi*4 + c_lo
HJ = 98                 # ji per half
FH = HJ * 4             # 392 moving cols per half (ji x r)
ROW = HH                # 17 floats per output row
SLAB = JI * HH          # 3332 floats per (b,c,o)

F32 = mybir.dt.float32
F32R = mybir.dt.float32r


def _build_nc():
    nc = bacc.Bacc()
    x_d = nc.dram_tensor("x3", [NB, CHI, 16, 784], F32R, kind="ExternalInput")
    xa_d = nc.dram_tensor("xa", [NB, CHI, 4, 196], F32R, kind="ExternalInput")
    w_d = nc.dram_tensor("w3", [16, CHI * 4 * 128], F32R, kind="ExternalInput")
    wa_d = nc.dram_tensor("wact", [4, 128], F32R, kind="ExternalInput")
    out_d = nc.dram_tensor("out", [NB, C, O * JI, HH], F32, kind="ExternalOutput")

    with TileContext(nc) as tc:
        with (
            tc.tile_pool(name="wpool", bufs=1) as wpool,
            tc.tile_pool(name="xpool", bufs=4) as xpool,
            tc.tile_pool(name="stage", bufs=6) as spool,
            tc.tile_pool(name="psv", bufs=6, space="PSUM") as pv_pool,
            tc.tile_pool(name="psa", bufs=2, space="PSUM") as pa_pool,
        ):
            # Resident weights.
            # w_sb rows 0..16 = (c_lo, q); free = (c_hi, p, 128 block-diag cols)
            wact_sb = wpool.tile([128, 128], F32R)
            nc.sync.dma_start(out=wact_sb[32:36, :], in_=wa_d[:, :])
            w_sb = wpool.tile([16, CHI * 4 * 128], F32R)
            nc.sync.dma_start(out=w_sb[:, 0:512], in_=w_d[:, 0:512])
            nc.sync.dma_start(out=w_sb[:, 512:], in_=w_d[:, 512:])

            HSLAB = HJ * ROW    # 1666 floats per half-slab row
            for b in range(NB):
                for c_hi in range(CHI):
                    x_sb = xpool.tile([128, 784], F32R, tag="x")
                    # pose rows (c_lo,q) at partitions 0..16  (HWDGE/ACT ring)
                    nc.scalar.dma_start(out=x_sb[0:16, :], in_=x_d[b, c_hi])
                    # act rows (c_lo) at partitions 32..36 (strip 1), compact ji
                    nc.scalar.dma_start(out=x_sb[32:36, 0:196], in_=xa_d[b, c_hi])

                    for half in range(2):
                        cl, ch = half * FH, (half + 1) * FH
                        stage = spool.tile([128, HSLAB], F32, tag="stage")
                        sview = stage.rearrange("z (ji t) -> z ji t", t=ROW)

                        at = pa_pool.tile([128, HJ], F32, tag="a")
                        nc.tensor.matmul(
                            at[:, :],
                            wact_sb[32:36, :],
                            x_sb[32:36, half * HJ:(half + 1) * HJ],
                        )
                        nc.vector.tensor_copy(sview[:, :, 16], at[:, :])

                        for p in range(4):
                            vt = pv_pool.tile([128, FH], F32, tag="v")
                            lhsT = w_sb[0:16,
                                        (c_hi * 4 + p) * 128:(c_hi * 4 + p + 1) * 128]
                            nc.tensor.matmul(
                                vt[:, :],
                                lhsT,
                                x_sb[0:16, cl:ch],
                            )
                            # Interleave-copy PSUM -> staging rows (ji*17 + p*4 + r)
                            src = vt.rearrange("z (jj r) -> z jj r", r=4)
                            dst = sview[:, :, p * 4: p * 4 + 4]
                            if p < 2:
                                nc.vector.tensor_copy(dst, src)
                            else:
                                nc.scalar.copy(dst, src)

                        # 0.85MB half out-DMA: dst [c_lo 4][o 32][1666 contig]
                        dst = out_d.rearrange(
                            "b (ch cl) (o j) t -> b ch cl o (j t)", cl=CLO, o=O
                        )[b, c_hi, :, :, half * HSLAB:(half + 1) * HSLAB]
                        nc.sync.dma_start(out=dst, in_=stage[:])
    if not nc.is_finalized():
        nc.finalize()
    return nc


_CACHE = {}


def _get_nc():
    if "nc" not in _CACHE:
        _CACHE["nc"] = _build_nc()
    return _CACHE["nc"]


def _preprocess(x, weight):
    """Build per-core input maps from full inputs."""
    x = np.ascontiguousarray(x, dtype=np.float32)
    weight = np.ascontiguousarray(weight, dtype=np.float32)
    xp = x.transpose(0, 1, 3, 2, 4).reshape(B, C, JI, HH)  # ji = J*14+I
    # x3[b, c_hi, (c_lo,q), (ji,r)] pose rows; xa[b, c_hi, c_lo, ji] acts
    pose = xp[..., :16].reshape(B, C, JI, 4, 4)            # [b,c,ji,q,r]
    x3p = np.ascontiguousarray(
        pose.transpose(0, 1, 3, 2, 4).reshape(B, CHI, 16, 784))
    xa = np.ascontiguousarray(xp[..., 16].reshape(B, CHI, 4, 196))

    Wm = weight[:, 0, 0]                                   # (C, O, 4, 4): W[c,o,p,q]
    w3 = np.zeros((16, CHI, 4, 128), dtype=np.float32)
    WmB = Wm.reshape(CHI, CLO, O, 4, 4)                    # [c_hi, c_lo, o, p, q]
    for c_lo in range(CLO):
        # dst w3[c_lo*4+q, c_hi, p, c_lo*32+o]
        w3[c_lo * 4:c_lo * 4 + 4, :, :, c_lo * 32:c_lo * 32 + 32] = (
            WmB[:, c_lo].transpose(3, 0, 2, 1))            # (q, c_hi, p, o)
    w3 = np.ascontiguousarray(w3.reshape(16, CHI * 4 * 128))

    wact = np.zeros((4, 128), dtype=np.float32)
    for c_lo in range(CLO):
        wact[c_lo, c_lo * 32:(c_lo + 1) * 32] = 1.0

    in_maps = []
    for k in range(NCORES):
        in_maps.append({
            "x3": np.ascontiguousarray(x3p[k * NB:(k + 1) * NB]),
            "xa": np.ascontiguousarray(xa[k * NB:(k + 1) * NB]),
            "w3": w3,
            "wact": wact,
        })
    return in_maps


def _run(x, weight, trace=False, trace_kwargs=None):
    nc = _get_nc()
    in_maps = _preprocess(x, weight)
    res = run_bass_kernel_spmd(
        nc, in_maps, list(range(NCORES)), trace=trace,
        trace_kwargs=trace_kwargs or {},
    )
    out = np.concatenate([r["out"] for r in res.results], axis=0)
    return out.astype(np.float32, copy=False), res


def kernel(x, weight):
    out, _ = _run(x, weight)
    return out
